# revision 53
# baseline (speedup 1.0000x reference)
"""Trainium2 Bass kernel for nn_PredictionModel (CPC-style prediction scores).

Reference computation (B=4, L=512, D=512, C=256, K=12, LW=500):
  cp[b,l,k,:]    = c[b,l,:] @ Wk[k].T            (row of R^D)
  zw[b,l,k,:]    = z[b, l+1+k, :]
  pos[b,l,k]     = <cp[b,l,k], zw[b,l,k]>
  neg_g[b,n,l,k] = <cp[b,l,k], zw[perm_B[n], perm_L[l], k]>
  neg_len[b,n,l,k]=<cp[b,l,k], zw[b, perms_len[n,l], k]>
  out = concat([pos[:,None], neg_g, neg_len], axis=1)   # (B, 9, LW, K)

Key algebraic move (C-space dots): <c[l] @ Wk[k].T, z[r]> = <c[l], z[r] @ Wk[k]>.
Define zp[r,k,:] = z[r,:] @ Wk[k] in R^C; every score is
  score[u, q, k] = <c[b_u, l_u(q), :], zp[b_src, q + k, k, :]>
with q = sigma_u(l) + 1 baked into a per-unit gather of c rows.

This version keeps the entire dot-product phase on the PE:
  - zpsT[c, q, k] is computed by matmuls directly in c-major layout
    (lhsT = Wk c-block, rhs = z^T k-shifted q-window).
  - the gathered c rows are pre-TRANSPOSED on the host (cgt[c, (q, u)]),
    so no on-chip transpose or per-row gather is needed.
  - per q-slot, ONE tiny matmul per c-chunk computes all 12x9 scores:
      lhsT = zpsT[:, q, :12k]  (12 cols), rhs = cgt[:, q, :9u] (9 cols)
      -> psum [12, 9], accumulated over the 2 c-chunks.
    PE streaming cost = 9 cols * 512 q-slots * 2 chunks ~ 4.6k cycles.

Per-core plan (8 cores = 4 source-batches x 2 q-halves): each core builds
zpsT for its 256 q-slots and scores all 9 units for them. Scheduling notes:
wk streams in 12 single-k DMAs in PE consumption order; the last zps k-pair
is computed q-block-major with per-(qj,cch) psum tiles so its SBUF copies
fire early and the score phase follows zps with no PE gap; scores drain
psum->sbuf in bank-sized chunks on alternating DVE/ACT and leave via two
output DMAs on separate queues. Scores are emitted q-indexed; the host
un-permutes (pure indexing).
"""

import numpy as np
import ml_dtypes

import concourse.mybir as mybir
from concourse import bacc
from concourse.tile import TileContext
from concourse import bass_utils

B, L, D, C, K = 4, 512, 512, 256, 12
LW = L - K            # 500
NM = 2 * B + 1        # 9 output channels
NU = 9                # units per source batch
NQB = 2               # q-blocks (of 128) per core
ZT_PAD = 272          # per-core z^T window: 256 + 16 (k-shift slack)
QCH = (0, 56, 112, 128)   # score psum chunks (<=56*9*4B = 2016B per bank)
F32 = mybir.dt.float32
BF16 = mybir.dt.bfloat16
BF16_NP = ml_dtypes.bfloat16

_NC = None

CFG = {
    "warmup": 34,      # PE p-state junk matmuls (128 cols each)
    "psz_bufs": 4,
    "copy_engines": ("v", "a"),   # round-robin for psum->sbuf copies (PSUM: DVE/ACT only)
}


def _build_program(cfg=None):
    """One NeuronCore program, identical across the 8 cores."""
    global _NC
    if cfg is None and _NC is not None:
        return _NC
    cfg = {**CFG, **(cfg or {})}

    nc = bacc.Bacc()
    # z[b_src]^T window: [128 d-part, 4 d-chunk, 272 r]
    zt_d = nc.dram_tensor("zt", [128, 4, ZT_PAD], BF16, kind="ExternalInput")
    # Wk transposed: [128 d-part, 4 d-chunk, K, C]
    wk_d = nc.dram_tensor("wk", [128, 4, K, C], BF16, kind="ExternalInput")
    # gathered+transposed c rows: [128 c-part, 2 qj, 2 cc, 128 s, 9 u]
    cgt_d = nc.dram_tensor("cgt", [128, NQB, 2, 128, NU], BF16, kind="ExternalInput")
    # scores: [12 k-part, 2 qj, 128 s, 9 u]
    out_d = nc.dram_tensor("out", [K, NQB, 128, NU], F32, kind="ExternalOutput")

    with TileContext(nc) as tc:
        with (
            tc.tile_pool(name="const", bufs=1) as const_pool,
            tc.tile_pool(name="psz", bufs=cfg["psz_bufs"], space="PSUM") as psz_pool,
            tc.tile_pool(name="pss", bufs=1, space="PSUM") as pss_pool,
        ):
            # PE p-state warmup: junk matmuls keep the tensor engine's busy
            # streak alive (bridging until the first wk chunk lands) so real
            # matmuls run at full clock.
            jmm = const_pool.tile([128, 128], BF16, name="jmm")
            nc.gpsimd.memset(jmm[:], 0.0)
            psj = psz_pool.tile([128, 2, 256], F32, name="psj", tag="z")
            for _ in range(cfg["warmup"]):
                nc.tensor.matmul(
                    psj[:, 0, :128], jmm[:], jmm[:], start=True, stop=True
                )
            del psj

            # Input DMAs on the SP queue, in consumption order: zt, then wk
            # in k-pair chunks (PE consumes k-sequentially), then cgt.
            zt_sb = const_pool.tile([128, 4, ZT_PAD], BF16, name="zt_sb")
            nc.sync.dma_start(out=zt_sb[:], in_=zt_d[:])
            wk_sb = const_pool.tile([128, 4, K, C], BF16, name="wk_sb")
            for j in range(K):
                nc.sync.dma_start(
                    out=wk_sb[:, :, j : j + 1], in_=wk_d[:, :, j : j + 1]
                )
            # cgt arrives last; qj1 is sub-split to unblock the final score
            # chunks as each slice lands.
            cgt_sb = const_pool.tile([128, NQB, 2, 128, NU], BF16, name="cgt_sb")
            for qj, ci in ((0, 0), (0, 1), (1, 0), (0, 2), (1, 1), (1, 2)):
                q0, q1 = QCH[ci], QCH[ci + 1]
                nc.sync.dma_start(
                    out=cgt_sb[:, qj, :, q0:q1], in_=cgt_d[:, qj, :, q0:q1]
                )

            # zpsT[c-part, qj, cc, q, k] built by PE in c-major layout; one
            # matmul group covers both q-blocks (rhs = 256-wide zt window).
            zps_sb = const_pool.tile([128, NQB, 2, 128, K], BF16, name="zps_sb")
            copy_engs = {
                "v": nc.vector.tensor_copy,
                "a": nc.scalar.copy,
            }
            ce_order = cfg["copy_engines"]
            n_copy = 0
            for g in range(K // 2 - 1):
                for cch in range(2):
                    ps = psz_pool.tile(
                        [128, 2, 256], F32, tag="z", name=f"psz{g}_{cch}"
                    )
                    for kk in range(2):
                        k = 2 * g + kk
                        for dc in range(4):
                            nc.tensor.matmul(
                                ps[:, kk],
                                wk_sb[:, dc, k, cch * 128 : (cch + 1) * 128],
                                zt_sb[:, dc, k : k + 256],
                                start=(dc == 0),
                                stop=(dc == 3),
                            )
                    eng = copy_engs[ce_order[n_copy % len(ce_order)]]
                    n_copy += 1
                    eng(
                        zps_sb[:, :, cch, :, 2 * g : 2 * g + 2],
                        ps[:].rearrange("p k (qj s) -> p qj s k", qj=NQB),
                    )
            # final k-pair: computed q-block-major (qj0 ranges first, both
            # c-chunks) so its copies -- which gate the whole score phase --
            # fire while the PE streams the qj1 half.
            g = K // 2 - 1
            for qj in range(NQB):
                s0, s1 = qj * 128, (qj + 1) * 128
                for cch in range(2):
                    # fresh tile per (qj, cch): sharing across qj would stall
                    # qj1's matmuls behind qj0's copy (writer-after-reader)
                    ps_l = psz_pool.tile(
                        [128, 2, 256], F32, tag="z", name=f"psz{g}_{cch}_{qj}"
                    )
                    for kk in range(2):
                        k = 2 * g + kk
                        for dc in range(4):
                            nc.tensor.matmul(
                                ps_l[:, kk, s0:s1],
                                wk_sb[:, dc, k, cch * 128 : (cch + 1) * 128],
                                zt_sb[:, dc, k + s0 : k + s1],
                                start=(dc == 0),
                                stop=(dc == 3),
                            )
                    eng = copy_engs["v" if cch == 0 else "a"]
                    eng(
                        zps_sb[:, qj, cch, :, 2 * g : 2 * g + 2],
                        ps_l[:, :, s0:s1].rearrange("p k s -> p s k"),
                    )

            # score phase: per q-slot, lhsT = zpsT (12 k-cols), rhs = cgt
            # (9 u-cols) -> psum [12, 9], accumulated over the 2 c-chunks.
            sc_sb = const_pool.tile([K, NQB, 128, NU], F32, name="sc_sb")
            for qj in range(NQB):
                for ci in range(len(QCH) - 1):
                    q0, q1 = QCH[ci], QCH[ci + 1]
                    qn = q1 - q0
                    # 4 psum tags rotated so no chunk ever waits on a drain:
                    # qj0 -> A,B,C ; qj1 -> D,A,C (A drains long before reuse)
                    tag = [["A", "B", "C"], ["D", "A", "C"]][qj][ci]
                    st = pss_pool.tile(
                        [K, qn, NU], F32, tag=f"st{tag}", name=f"st{qj}_{ci}"
                    )
                    for s in range(qn):
                        for cch in range(2):
                            nc.tensor.matmul(
                                st[:, s, :],
                                zps_sb[:, qj, cch, q0 + s, :],
                                cgt_sb[:, qj, cch, q0 + s, :],
                                start=(cch == 0),
                                stop=(cch == 1),
                            )
                    # whole-chunk drains, queues balanced so the final small
                    # chunk never waits behind a 56-wide drain
                    eng = copy_engs[[["v", "a", "v"], ["a", "v", "a"]][qj][ci]]
                    eng(sc_sb[:, qj, q0:q1, :], st[:])
                out_eng = nc.scalar if qj == 0 else nc.sync
                out_eng.dma_start(out=out_d[:, qj], in_=sc_sb[:, qj])

    nc.compile()
    if cfg == CFG:
        _NC = nc
    return nc


def _unit_perms(perms_len, perm_L, perm_B, b_src):
    """Per-unit (b_out, channel, forward-perm sl(l), inverse-perm l(sl))."""
    ident = np.arange(LW)
    inv_len = [np.argsort(perms_len[n]) for n in range(B)]
    inv_pl = np.argsort(perm_L)
    n_src = int(np.nonzero(perm_B == b_src)[0][0])
    units = [(b_src, 0, ident, ident)]
    for n in range(B):
        units.append((b_src, 1 + B + n, perms_len[n], inv_len[n]))
    for b_out in range(B):
        units.append((b_out, 1 + n_src, perm_L, inv_pl))
    return units


def _make_inputs(c, z, Wk, perms_len, perm_L, perm_B):
    """Host-side prep: transposed/padded operands + pre-transposed c gather."""
    c_bf = c.astype(BF16_NP)  # [B, L, C]
    wk_dc = np.ascontiguousarray(
        Wk.reshape(K, 4, 128, C).transpose(2, 1, 0, 3)
    ).astype(BF16_NP)  # [128 dp, 4 dc, K, C]

    zt_full = np.zeros((B, 128, 4, L + 16), dtype=BF16_NP)
    for b in range(B):
        # zt[dp, dc, r] = z[b, r, dc*128+dp]
        zt = z[b].T.reshape(4, 128, L).transpose(1, 0, 2)  # [128, 4, 512]
        zt_full[b, :, :, :L] = zt.astype(BF16_NP)

    in_maps = []
    s_all = np.arange(NQB * 128)
    for b_src in range(B):
        units = _unit_perms(perms_len, perm_L, perm_B, b_src)
        for g in range(2):
            ztw = np.ascontiguousarray(
                zt_full[b_src, :, :, g * 256 : g * 256 + ZT_PAD]
            )
            # cg[u, s, c] = c[b_u, inv_u(q_glob - 1), c], 0 where invalid
            cg = np.zeros((NU, NQB * 128, C), dtype=BF16_NP)
            for u, (b_out, _ch, _fwd, inv) in enumerate(units):
                q_glob = g * 256 + s_all
                sl = q_glob - 1
                valid = (sl >= 0) & (sl < LW)
                cg[u, valid] = c_bf[b_out][inv[sl[valid]]]
            # -> cgt[cp, qj, cc, s, u]
            cgt = np.ascontiguousarray(
                cg.reshape(NU, NQB, 128, 2, 128).transpose(4, 1, 3, 2, 0)
            )
            in_maps.append({"zt": ztw, "wk": wk_dc, "cgt": cgt})
    return in_maps


def kernel(c, z, Wk, perms_len, perm_L, perm_B, _trace=False, _result_holder=None):
    c = np.asarray(c, np.float32)
    z = np.asarray(z, np.float32)
    Wk = np.asarray(Wk, np.float32)
    perms_len = np.asarray(perms_len, np.int64)
    perm_L = np.asarray(perm_L, np.int64)
    perm_B = np.asarray(perm_B, np.int64)

    nc = _build_program()
    in_maps = _make_inputs(c, z, Wk, perms_len, perm_L, perm_B)
    res = bass_utils.run_bass_kernel_spmd(
        nc, in_maps, core_ids=list(range(2 * B)), trace=_trace
    )
    if _result_holder is not None:
        _result_holder.append(res)

    out = np.empty((B, NM, LW, K), np.float32)
    larr = np.arange(LW)
    for b_src in range(B):
        units = _unit_perms(perms_len, perm_L, perm_B, b_src)
        for g in range(2):
            co = res.results[2 * b_src + g]["out"]  # [K, NQB, 128, NU]
            for u, (b_out, ch, fwd, _inv) in enumerate(units):
                q = fwd + 1
                sel = (q // 256) == g
                qj = (q[sel] // 128) % 2
                s = q[sel] % 128
                out[b_out, ch, larr[sel]] = co[:, qj, s, u].T
    return out
